# revision 8
# baseline (speedup 1.0000x reference)
"""Chamfer loss kernel for 8 Trainium2 NeuronCores.

Strategy
--------
Data parallel over the batch dim: B=16 point clouds, 2 per core.

Host-side (cheap, O(B*K)): compact each cloud to its valid points (the
reference masks invalid rows/cols out of the min with +inf, and the same
mask applies to both sides, so dropping invalid points is exact). Pad to a
common K_p (multiple of 128) with a far-away sentinel point P0=(1e4,1e4,1e4)
shared by pred and target: a padded pred row's nearest target is the padded
target at distance exactly 0 (contributes sqrt(1e-12)=1e-6, subtracted on the
host), and no real point ever selects a pad (d2 ~ 3e8).

Device-side (the O(B*K^2) work): for each batch, d2[i,j] is produced by the
TensorEngine as a single matmul with the squared norms folded in as extra
contraction rows:
    lhsT = [-2*px; -2*py; -2*pz; x2; 1]   (5 x K_p, stationary per row-block)
    rhs  = [ tx ;  ty ;  tz ; 1 ; y2]     (5 x K_p, moving)
    psum[i,j] = x2[i] + y2[j] - 2*<p_i, t_j>  (fp32, exact-ish)
ScalarE casts each PSUM chunk to bf16 in SBUF. VectorE then computes, per
chunk, the row-direction running min via tensor_scalar (op0=max(.,1e-12)
clamp, op1=min free-dim reduce, 4x bf16 mode) and the column-direction
elementwise min accumulation via tensor_tensor(min) (2x bf16 mode). The
column accumulator is reduced across partitions by PE-transposing each
128-wide block (identity matmul) and min-reducing the transposed block's
free dim with tensor_scalar. The host finishes sqrt and sums on the tiny
per-row/per-column minima vectors.
"""

import math

import numpy as np

import concourse.bass as bass
import concourse.tile as tile
from concourse import mybir
from concourse.bass_utils import run_bass_kernel_spmd

N_CORES = 8
B, K, D = 16, 4096, 3
PAD_COORD = 1.0e4
BIG = 1.0e30
CLAMP = 1.0e-12

F32 = mybir.dt.float32
BF16 = mybir.dt.bfloat16


# ---------------------------------------------------------------------------
# walrus workaround: this build has small per-instruction sync-wait slot
# budgets (1 for Drain, ~2 for LDWEIGHTS etc). Move excess waits onto
# preceding NOPs on the same engine.
def _split_excess_waits(nc, default_max: int = 1, per_type: dict | None = None):
    per_type = per_type or {"InstDrain": 1, "InstMatmult": 1}
    for _bbname, bbobj in list(nc.bb_map.items()):
        inner = bbobj.bb
        insts = inner.instructions
        i = 0
        while i < len(insts):
            inst = insts[i]
            si = inst.sync_info
            max_waits = per_type.get(type(inst).__name__, default_max)
            if (
                si is not None
                and si.on_wait
                and len(si.on_wait) > max_waits
            ):
                waits = list(si.on_wait)
                keep, extra = waits[:max_waits], waits[max_waits:]
                eng = nc.engines[inst.engine]
                new_nops = []
                for w in extra:
                    eng.nop()
                    src = nc.cur_bb.bb.instructions
                    raw = src[-1]
                    assert type(raw).__name__ == "InstNoOp", type(raw).__name__
                    del src[-1]
                    raw.sync_info = mybir.SyncInfo(on_wait=[w], on_update=[])
                    new_nops.append(raw)
                inst.sync_info = mybir.SyncInfo(
                    on_wait=keep, on_update=list(si.on_update or [])
                )
                for j, nop in enumerate(new_nops):
                    insts.insert(i + j, nop)
                i += len(new_nops)
            i += 1


def _chunks_of(width: int):
    """Column chunks: as many 1024-wide (2 PSUM banks) as fit + remainder."""
    out = []
    c0 = 0
    while width - c0 >= 1024:
        out.append((c0, 1024))
        c0 += 1024
    if width - c0 > 0:
        out.append((c0, width - c0))
    return out


def build_nc(K_p: int, n_batches: int = 2):
    RB = K_p // 128
    chunks = _chunks_of(K_p)
    NCH = len(chunks)

    nc = bass.Bass("TRN2", target_bir_lowering=False, debug=False, num_devices=1)

    mats_in = []
    for b in range(n_batches):
        L = nc.dram_tensor(f"L{b}", [5, K_p], F32, kind="ExternalInput")
        R = nc.dram_tensor(f"R{b}", [5, K_p], F32, kind="ExternalInput")
        mats_in.append((L, R))

    ident_in = nc.dram_tensor("ident", [128, 128], BF16, kind="ExternalInput")
    rowparts_d = nc.dram_tensor(
        "rowparts", [128, n_batches * RB * NCH], F32, kind="ExternalOutput"
    )
    colmins_d = nc.dram_tensor(
        "colmins", [128, n_batches * RB], F32, kind="ExternalOutput"
    )

    amax = mybir.AluOpType.max
    amin = mybir.AluOpType.min

    with tile.TileContext(nc) as tc:
        with (
            tc.tile_pool(name="consts", bufs=1) as consts,
            tc.tile_pool(name="work", bufs=4) as work,
            tc.tile_pool(name="psA", bufs=2, space="PSUM") as psA,
            tc.tile_pool(name="psB", bufs=2, space="PSUM") as psB,
        ):
            # load the per-batch augmented matrices
            LR = []
            for b in range(n_batches):
                Lt = consts.tile([5, K_p], F32, tag=f"L{b}")
                nc.sync.dma_start(Lt[:], mats_in[b][0].ap())
                Rt = consts.tile([5, K_p], F32, tag=f"R{b}")
                nc.sync.dma_start(Rt[:], mats_in[b][1].ap())
                LR.append((Lt, Rt))

            rowparts_sb = consts.tile([128, n_batches * RB * NCH], F32, tag="rp")
            colmins_sb = consts.tile([128, n_batches * RB], F32, tag="cm")
            ident = consts.tile([128, 128], BF16, tag="ident")
            nc.sync.dma_start(ident[:], ident_in.ap())

            for b in range(n_batches):
                Lt, Rt = LR[b]
                colacc = consts.tile([128, K_p], BF16, tag=f"colacc{b}")
                nc.vector.memset(colacc[:], BIG)

                for ib in range(RB):
                    lhsT = Lt[:, ib * 128 : (ib + 1) * 128]
                    for ci, (c0, cw) in enumerate(chunks):
                        pool = psA if cw > 512 else psB
                        ps = pool.tile([128, cw], F32, tag=f"ps{cw}")
                        for s in range(0, cw, 512):
                            w = min(512, cw - s)
                            nc.tensor.matmul(
                                ps[:, s : s + w],
                                lhsT,
                                Rt[:, c0 + s : c0 + s + w],
                                start=True,
                                stop=True,
                            )
                        sb = work.tile([128, cw], BF16, tag=f"sb{cw}")
                        nc.scalar.copy(sb[:], ps[:])
                        sb2 = work.tile([128, cw], BF16, tag=f"sb2{cw}")
                        idx = (b * RB + ib) * NCH + ci
                        rp = rowparts_sb[:, idx : idx + 1]
                        nc.vector.tensor_scalar(
                            sb2[:], sb[:], CLAMP, None, amax, amin, accum_out=rp
                        )
                        nc.vector.tensor_tensor(
                            colacc[:, c0 : c0 + cw], sb[:], colacc[:, c0 : c0 + cw], amin
                        )

                for ib in range(RB):
                    tp = psB.tile([128, 128], BF16, tag="trp")
                    nc.tensor.transpose(
                        tp[:], colacc[:, ib * 128 : (ib + 1) * 128], ident[:]
                    )
                    tg = work.tile([128, 128], BF16, tag="tg")
                    cm = colmins_sb[:, b * RB + ib : b * RB + ib + 1]
                    nc.vector.tensor_scalar(
                        tg[:], tp[:], CLAMP, None, amax, amin, accum_out=cm
                    )

            nc.sync.dma_start(rowparts_d.ap(), rowparts_sb[:])
            nc.sync.dma_start(colmins_d.ap(), colmins_sb[:])

    _split_excess_waits(nc)
    return nc, RB, chunks


def _host_prep(pred, target, mask):
    """Compact+pad each batch; build the augmented 5 x K_p matrices."""
    counts = mask.sum(axis=1).astype(np.int64)
    K_p = max(128, int(math.ceil(counts.max() / 128.0)) * 128)

    Ls = np.empty((B, 5, K_p), np.float32)
    Rs = np.empty((B, 5, K_p), np.float32)
    for b in range(B):
        n = int(counts[b])
        p = np.full((K_p, 3), PAD_COORD, np.float32)
        t = np.full((K_p, 3), PAD_COORD, np.float32)
        p[:n] = pred[b][mask[b]]
        t[:n] = target[b][mask[b]]
        x2 = (p * p).sum(axis=1, dtype=np.float32)
        y2 = (t * t).sum(axis=1, dtype=np.float32)
        Ls[b, 0:3] = (-2.0 * p).T
        Ls[b, 3] = x2
        Ls[b, 4] = 1.0
        Rs[b, 0:3] = t.T
        Rs[b, 3] = 1.0
        Rs[b, 4] = y2
    return counts, K_p, Ls, Rs


_NC_CACHE = {}

try:
    import ml_dtypes

    _IDENT = np.eye(128, dtype=ml_dtypes.bfloat16)
except ImportError:  # pragma: no cover
    _IDENT = np.eye(128, dtype=np.float32)


def kernel(pred, target, mask):
    pred = np.asarray(pred, np.float32)
    target = np.asarray(target, np.float32)
    mask = np.asarray(mask).astype(bool)

    counts, K_p, Ls, Rs = _host_prep(pred, target, mask)
    nb = B // N_CORES  # batches per core

    key = (K_p, nb)
    if key not in _NC_CACHE:
        _NC_CACHE[key] = build_nc(K_p, nb)
    nc, RB, chunks = _NC_CACHE[key]
    NCH = len(chunks)

    in_maps = []
    for c in range(N_CORES):
        m = {}
        for j in range(nb):
            m[f"L{j}"] = Ls[c * nb + j]
            m[f"R{j}"] = Rs[c * nb + j]
        m["ident"] = _IDENT
        in_maps.append(m)

    res = run_bass_kernel_spmd(nc, in_maps, core_ids=list(range(N_CORES)))

    total = np.float32(counts.sum())
    s = np.float64(0.0)
    for c in range(N_CORES):
        rowparts = np.asarray(res.results[c]["rowparts"], np.float32)
        colmins = np.asarray(res.results[c]["colmins"], np.float32)
        for j in range(nb):
            n = int(counts[c * nb + j])
            # row direction: [128, RB, NCH] -> min over chunks
            rp = rowparts[:, (j * RB * NCH) : ((j + 1) * RB * NCH)]
            rp = rp.reshape(128, RB, NCH).min(axis=2)  # [128, RB]
            rowmin = rp.T.reshape(-1)[:n]  # row r = ib*128+p -> [RB,128]
            s += np.sqrt(np.maximum(rowmin, CLAMP), dtype=np.float32).sum(dtype=np.float64)
            # col direction: colmins[p, j*RB+ib] = min for column ib*128+p
            ct = colmins[:, j * RB : (j + 1) * RB].T.reshape(-1)[:n]
            s += np.sqrt(np.maximum(ct, CLAMP), dtype=np.float32).sum(dtype=np.float64)

    loss = s / (2.0 * (np.float64(total) + 1e-8))
    return np.float32(loss)
